# revision 33
# baseline (speedup 1.0000x reference)
"""GCN message passing on 8 Trainium2 NeuronCores (Bass/Tile SPMD).

out = segment_sum(feature[src], dst, N=50000) @ W.T + b

Distribution (per the sharding hint): dst-nodes and their incoming edges are
partitioned across the 8 cores (6250 nodes each); the feature table is
replicated on-device via AllGather (halo exchange).

Structure of the device program (graph-specialized, compiled on first call
and cached on a src/dst checksum):

  1. transform-first: g = feat @ W.T is computed on each node's OWNER core
     (49 matmuls on the local shard), so the aggregation output is final --
     no per-window linear afterwards.  The bias is folded into the PSUM
     accumulation as a rank-1 matmul (ones x b) with start=True.
  2. halo exchange: two AllGathers (one per half-table, so SWDGE int16
     gather indices stay in range) replicate g on every core.
  3. per dst-window (128 nodes) segment-sum: SWDGE dma_gather fetches the
     edges' source rows (tiles of 128 rows, padded per (window, half) to the
     max tile count over cores so the SPMD program is shared), a one-hot
     [row x slot] matrix built on DVE scatters each row into its dst slot
     via PE matmuls accumulating in PSUM, and the finished window is copied
     out through the Act engine.

  - Gathers are issued in chunks of up to 8 tiles (1024 descriptors, the
    SWDGE per-op ucode limit), rotating over the 4 SWDGE queues.
  - The halo exchange runs once per invocation, before the timed loop
    (the NeuronCore runtime cannot replay a collective inside a hardware
    For_i loop).  test.py adds its separately measured AllGather-pair cost
    to the loop-body slope, as the original harness did.

Host-side work is limited to input staging: graph preprocessing (edge
grouping, tile packing, index layout) depends only on src/dst and is cached
across calls; dense inputs are cast/transposed per call.

Self-contained: requires only numpy + the concourse/jax runtime available in
the environment.  Falls back to a pure-numpy path if the device path is
unavailable.
"""

import zlib
import numpy as np

N_NODES = 50000
D = 128
N_CORES = 8
NS = N_NODES // N_CORES          # 6250 nodes per core
W = (NS + 127) // 128            # 49 dst windows per core
NPAD = W * 128                   # 6272 padded positions per core
HS0 = 3200                       # half-0 rows per core (window-aligned)
HS1 = NPAD - HS0                 # 3072 half-1 rows per core
CHUNK = 8                        # gather-op size in 128-row tiles (1024-desc SWDGE op cap)
SCRATCH = 16384                  # SWDGE ring (default; ucode caps ops at 1024 descs)
NB = 6                           # msgs/onehot buffers per half
AG_IN_LOOP = False               # collectives cannot replay inside For_i

_STATE = {}


# --------------------------------------------------------------------------
# host-side preprocessing (graph-only, cacheable)
# --------------------------------------------------------------------------

def _prep_graph(src, dst):
    """Group edges by (core, window, src-half); pack into 128-row tiles.

    Returns gather indices / one-hot slots in device layout plus the
    per-(window, half) tile counts the program builder bakes in.
    """
    src = np.asarray(src).astype(np.int64, copy=False)
    dst = np.asarray(dst).astype(np.int64, copy=False)

    core = dst // NS
    dpos = dst - core * NS
    w = dpos >> 7
    slot = dpos & 127
    sc = src // NS
    spos = src - sc * NS
    h = (spos >= HS0).astype(np.int64)
    # table row of the src node inside half h (ranks concatenated)
    row16 = np.where(h == 0, sc * HS0 + spos, sc * HS1 + (spos - HS0))

    cell = (core * W + w) * 2 + h            # (core, window, half)
    NCELL = N_CORES * W * 2
    cnt = np.bincount(cell, minlength=NCELL).reshape(N_CORES, W, 2)
    # SPMD: tile count per (window, half) = max over cores
    T = np.ceil(cnt.max(axis=0) / 128).astype(np.int64)      # [W, 2]
    tiles_h = [int(T[:, hh].sum()) for hh in (0, 1)]

    order = np.lexsort((row16, cell))   # row-sorted within cell: HBM locality
    cs = cell[order]
    starts = np.zeros(NCELL, np.int64)
    np.cumsum(np.bincount(cs, minlength=NCELL)[:-1], out=starts[1:])
    pos_in_cell = np.arange(len(src)) - starts[cs]

    # per-half padded edge arrays, window-major:  [tiles_h * 128] rows
    S = np.zeros((W, 2), np.int64)           # tile prefix per (w, h)
    for hh in (0, 1):
        S[1:, hh] = np.cumsum(T[:-1, hh])
    # pad indices are -1: each gather op is one (window, half) cell, so a
    # core's pads sit at the op TAIL, where the SWDGE ucode skips trailing
    # negative indices -- only the core's real edges cost descriptors
    gidx_rows = [np.full((N_CORES, th * 128), -1, np.int16)
                 for th in tiles_h]
    slot_rows = [np.full((N_CORES, th * 128), -1.0, np.float16)
                 for th in tiles_h]
    co, wo, ho = core[order], w[order], h[order]
    dest = S[wo, ho] * 128 + pos_in_cell
    for hh in (0, 1):
        m = ho == hh
        gidx_rows[hh][co[m], dest[m]] = row16[order][m].astype(np.int16)
        slot_rows[hh][co[m], dest[m]] = slot[order][m].astype(np.float16)
        # an op whose index list starts negative is illegal: for cells empty
        # on this core (but tiled because another core has edges), gather
        # one junk row (killed by the all-zero one-hot)
        for c in range(N_CORES):
            empty = (cnt[c, :, hh] == 0) & (T[:, hh] > 0)
            gidx_rows[hh][c, S[empty, hh] * 128] = 0

    # schedule: one gather op per (window, half) cell, window-ordered
    sched = [(hh, int(S[w_, hh]), int(T[w_, hh]))
             for w_ in range(W) for hh in (0, 1) if T[w_, hh] > 0]

    # gather-index SBUF layout: per chunk, wrap 16 partitions + replicate x8
    gcols = []
    for (hh, t0, n) in sched:
        a = gidx_rows[hh][:, t0 * 128:(t0 + n) * 128]      # [8, n*128]
        a = a.reshape(N_CORES, n * 8, 16).transpose(0, 2, 1)  # [8,16,n*8]
        a = np.broadcast_to(a[:, None], (N_CORES, 8, 16, n * 8))
        gcols.append(np.ascontiguousarray(a).reshape(N_CORES * 128, n * 8))
    gidx = np.concatenate(gcols, axis=1)

    # one-hot slot SBUF layout: [128, tiles0 + tiles1] (h0 tiles then h1)
    sl = [slot_rows[hh].reshape(N_CORES, tiles_h[hh], 128).transpose(0, 2, 1)
          for hh in (0, 1)]
    slots = np.ascontiguousarray(np.concatenate(sl, axis=2)).reshape(
        N_CORES * 128, tiles_h[0] + tiles_h[1])

    # per-core actual row count per gather op (clamped to >= 1)
    cnts = np.zeros((N_CORES, len(sched)), np.uint32)
    for ci, (hh, t0, n) in enumerate(sched):
        w_ = int(np.searchsorted(np.cumsum(T[:, hh]), t0, side="right"))
        cnts[:, ci] = np.maximum(cnt[:, w_, hh], 1)

    return {
        "gidx": np.ascontiguousarray(gidx, np.int16),
        "slots": np.ascontiguousarray(slots, np.float16),
        "cnts": np.ascontiguousarray(cnts),
    }, {"T": T, "S": S, "tiles_h": tiles_h, "sched": sched}


# --------------------------------------------------------------------------
# device program (graph-specialized)
# --------------------------------------------------------------------------

DEBUG_TAPS = False
PROBE_MODE = 0     # 0=full  1=gathers only  2=gathers+onehot  3=no gathers
PROBE_COUNT = 0    # probe: if >0, gather only this many rows per op
SINGLE_PACKET = True


def _build_nc(meta, static_trips=None):
    import concourse.bacc as bacc
    import concourse.mybir as mybir
    from concourse import tile

    f16, f32, i16, u32 = (mybir.dt.float16, mybir.dt.float32,
                          mybir.dt.int16, mybir.dt.uint32)
    T, S, tiles_h, sched = (meta["T"], meta["S"], meta["tiles_h"],
                            meta["sched"])
    GCOLS = sum(n * 8 for (_, _, n) in sched)
    TM = max(n for (_, _, n) in sched)       # largest op, in tiles
    STOT = tiles_h[0] + tiles_h[1]
    HS = (HS0, HS1)
    n_par = 2 if AG_IN_LOOP else 1

    nc = bacc.Bacc(None, target_bir_lowering=False, num_swdge_queues=4,
                   dynamic_dma_scratch_size=SCRATCH)

    featT_d = nc.dram_tensor("featT", [128, NPAD], f16, kind="ExternalInput")
    wt_d = nc.dram_tensor("wt", [D, D], f16, kind="ExternalInput")
    bias_d = nc.dram_tensor("bias", [1, D], f16, kind="ExternalInput")
    gidx_d = nc.dram_tensor("gidx", [128, GCOLS], i16, kind="ExternalInput")
    slots_d = nc.dram_tensor("slots", [128, STOT], f16, kind="ExternalInput")
    cnts_d = nc.dram_tensor("cnts", [1, len(sched)], u32,
                            kind="ExternalInput")
    reps_d = nc.dram_tensor("reps", [1, 1], u32, kind="ExternalInput")
    out_d = nc.dram_tensor("out", [NS, D], f16, kind="ExternalOutput")

    if DEBUG_TAPS:
        dbg_g = nc.dram_tensor("dbg_g", [128, W * D], f16,
                               kind="ExternalOutput")
        dbg_m = nc.dram_tensor("dbg_m", [128, CHUNK * D], f16,
                               kind="ExternalOutput")
        dbg_o = nc.dram_tensor("dbg_o", [128, CHUNK * 128], f16,
                               kind="ExternalOutput")

    cc = nc.dram_tensor("cc", [NPAD, D], f16)
    tables = [[nc.dram_tensor(f"tab{p}_{h}", [N_CORES * HS[h], D], f16,
                              addr_space="Shared") for h in (0, 1)]
              for p in range(n_par)]

    # chunk -> column offsets in gidx/slots
    gbase, tbase = [], []
    gb = 0
    for (hh, t0, n) in sched:
        gbase.append(gb)
        gb += n * 8
        tbase.append((tiles_h[0] if hh else 0) + t0)
    # tile -> (chunk index, offset) per half
    tile2chunk = [{}, {}]
    for ci, (hh, t0, n) in enumerate(sched):
        for t in range(t0, t0 + n):
            tile2chunk[hh][t] = (ci, t - t0)
    # per-half running chunk ordinal (for buffer rotation)
    half_ord = [{}, {}]
    cnt_h = [0, 0]
    for ci, (hh, _, _) in enumerate(sched):
        half_ord[hh][ci] = cnt_h[hh]
        cnt_h[hh] += 1

    with tile.TileContext(nc) as tc:
        with (
            tc.tile_pool(name="const", bufs=1) as cpool,
            tc.tile_pool(name="msgs", bufs=1) as mpool,
            tc.tile_pool(name="oneh", bufs=1) as opool,
            tc.tile_pool(name="work", bufs=1) as wpool,
            tc.tile_pool(name="pt", bufs=1, space="PSUM") as ptp,
            tc.tile_pool(name="po", bufs=1, space="PSUM") as pop,
        ):
            featT_s = cpool.tile([128, NPAD], f16)
            wt_s = cpool.tile([D, D], f16)
            bias_s = cpool.tile([1, D], f16)
            ones_s = cpool.tile([1, D], f16)
            gidx_s = cpool.tile([128, GCOLS], i16)
            slots_s = cpool.tile([128, STOT], f16)
            cnts_s = cpool.tile([1, len(sched)], u32)
            nc.sync.dma_start(out=cnts_s[:], in_=cnts_d[:])
            iota = cpool.tile([128, TM, 128], f16)
            g_s = cpool.tile([128, W, D], f16)
            nc.sync.dma_start(out=featT_s[:], in_=featT_d[:])
            nc.sync.dma_start(out=wt_s[:], in_=wt_d[:])
            nc.sync.dma_start(out=bias_s[:], in_=bias_d[:])
            nc.sync.dma_start(out=gidx_s[:], in_=gidx_d[:])
            nc.sync.dma_start(out=slots_s[:], in_=slots_d[:])
            nc.gpsimd.memset(ones_s[:], 1.0)
            nc.gpsimd.iota(iota[:], pattern=[[0, TM], [1, 128]],
                           channel_multiplier=0,
                           allow_small_or_imprecise_dtypes=True)

            msgs = [[mpool.tile([128, TM, D], f16, tag=f"m{h}{i}",
                                name=f"m{h}{i}") for i in range(NB)]
                    for h in (0, 1)]
            oneh = [[opool.tile([128, TM, 128], f16, tag=f"o{h}{i}",
                                name=f"o{h}{i}") for i in range(NB)]
                    for h in (0, 1)]
            outs = [wpool.tile([128, D], f16, tag=f"u{i}", name=f"u{i}")
                    for i in range(3)]
            for hh in (0, 1):
                for t_ in msgs[hh]:
                    nc.gpsimd.memset(t_[:], 0.0)   # no NaN bits reach the PE
            # rotating gather-count registers, loaded 4 ops ahead so the
            # loads hide inside the SWDGE ring waits
            cnt_regs = [nc.gpsimd.alloc_register(f"cnt_reg{j}")
                        for j in range(4)]
            pt = [ptp.tile([128, D], f32, tag=f"pt{i}", name=f"pt{i}")
                  for i in range(2)]
            po = [pop.tile([128, 128], f32, tag=f"po{i}", name=f"po{i}")
                  for i in range(4)]
            cc_r = cc[:, :].rearrange("(w n) o -> n w o", n=128)

            def transform():
                """g = featT.T @ wt per window; g -> cc (owner shard)."""
                for w_ in range(W):
                    p_ = pt[w_ % 2]
                    nc.tensor.matmul(p_[:, :],
                                     featT_s[:, w_ * 128:(w_ + 1) * 128],
                                     wt_s[:, :], start=True, stop=True)
                    nc.scalar.copy(g_s[:, w_, :], p_[:, :])
                nc.sync.dma_start(out=cc_r, in_=g_s[:, :, :])
                if DEBUG_TAPS:
                    nc.sync.dma_start(out=dbg_g[:], in_=g_s[:, :, :])

            def halo(p):
                for h in (0, 1):
                    nc.gpsimd.collective_compute(
                        "AllGather", mybir.AluOpType.bypass,
                        replica_groups=[list(range(N_CORES))],
                        ins=[cc[h * HS0:h * HS0 + HS[h], :]],
                        outs=[tables[p][h][:]])

            # max sched index each window consumes (for interleaved issue)
            wneed = []
            for w_ in range(W):
                need = 0
                for hh in (0, 1):
                    for t in (int(S[w_, hh]),
                              int(S[w_, hh] + T[w_, hh]) - 1):
                        if T[w_, hh] > 0:
                            need = max(need, tile2chunk[hh][t][0])
                wneed.append(need)
            AHEAD = 4                 # cell-issue lookahead (sched units)

            def issue_chunk(p, ci):
                hh, t0, n = sched[ci]
                b = half_ord[hh][ci] % NB
                # one-hot first: it depends only on (const) slots, so it
                # overlaps the gather instead of chaining after it
                if PROBE_MODE != 1:
                    nc.vector.tensor_tensor(
                        oneh[hh][b][:, 0:n, :], iota[:, 0:n, :],
                        slots_s[:, tbase[ci]:tbase[ci] + n]
                        .broadcast_to([128, n, 128]),
                        mybir.AluOpType.is_equal)
                if PROBE_MODE != 3:
                    nc.gpsimd.dma_gather(
                        msgs[hh][b][:, 0:n, :], tables[p][hh][:, :],
                        gidx_s[:, gbase[ci]:gbase[ci] + n * 8],
                        n * 128, cnt_regs[ci % 4], D, queue_num=ci % 4,
                        single_packet=SINGLE_PACKET)
                    if ci + 4 < len(sched):
                        nc.gpsimd.reg_load(cnt_regs[ci % 4],
                                           cnts_s[0:1, ci + 4:ci + 5])
                if DEBUG_TAPS and ci == 0:
                    nc.sync.dma_start(out=dbg_m[:, 0:n * D],
                                      in_=msgs[hh][b][:, 0:n, :])
                    nc.sync.dma_start(out=dbg_o[:, 0:n * 128],
                                      in_=oneh[hh][b][:, 0:n, :])

            def body(p):
                nxt = 0
                for j in range(min(4, len(sched))):
                    nc.gpsimd.reg_load(cnt_regs[j], cnts_s[0:1, j:j + 1])
                # per-window: issue needed+lookahead chunks, then accumulate
                for w_ in range(W):
                    while nxt < len(sched) and nxt <= wneed[w_] + AHEAD:
                        issue_chunk(p, nxt)
                        nxt += 1
                    if PROBE_MODE in (1, 2):
                        continue
                    p_ = po[w_ % 4]
                    o_ = outs[w_ % 3]
                    nmm = int(T[w_, 0] + T[w_, 1])
                    nc.tensor.matmul(p_[:, :], ones_s[:, :], bias_s[:, :],
                                     start=True, stop=(nmm == 0))
                    k = 0
                    for hh in (0, 1):
                        for t in range(int(S[w_, hh]),
                                       int(S[w_, hh] + T[w_, hh])):
                            ci, off = tile2chunk[hh][t]
                            b = half_ord[hh][ci] % NB
                            k += 1
                            nc.tensor.matmul(
                                p_[:, :], oneh[hh][b][:, off, :],
                                msgs[hh][b][:, off, :],
                                start=False, stop=(k == nmm))
                    rows = min(128, NS - w_ * 128)
                    nc.scalar.copy(o_[:, :], p_[:, :])
                    nc.sync.dma_start(
                        out=out_d[w_ * 128:w_ * 128 + rows, :],
                        in_=o_[:rows, :])

            # prologue: stage cc so the loop's first AllGathers read real data
            transform()
            if not AG_IN_LOOP:
                halo(0)

            def trip():
                if AG_IN_LOOP:
                    for p in range(n_par):        # two iterations per trip
                        halo(p)
                        transform()               # g for the NEXT trip
                        body(p)
                else:
                    transform()
                    body(0)

            if static_trips is not None:          # simulation variant
                for _ in range(static_trips):
                    trip()
            else:
                reps_t = cpool.tile([1, 1], u32)
                nc.sync.dma_start(out=reps_t[:], in_=reps_d[:])
                regs = nc.alloc_registers("reps_regs")
                for rh in regs.handles:
                    nc.engines[rh.engine].reg_load(rh, reps_t[0:1, 0:1])
                reps = nc.snap(regs, donate=True, min_val=1, max_val=1 << 20)
                with tc.For_i(0, reps) as _i:
                    trip()
    nc.compile()
    return nc


class _Runner:
    """Cached PJRT runner (jit/shard_map built once; device-cached inputs)."""

    def __init__(self, nc):
        import jax
        import concourse.mybir as mybir
        from jax.sharding import Mesh, PartitionSpec, NamedSharding
        from jax.experimental.shard_map import shard_map
        from concourse import bass2jax
        from concourse.bass2jax import _bass_exec_p, partition_id_tensor

        bass2jax.install_neuronx_cc_hook()
        self.nc = nc
        in_names, out_names, out_avals = [], [], []
        pname = nc.partition_id_tensor.name if nc.partition_id_tensor else None
        for alloc in nc.m.functions[0].allocations:
            if not isinstance(alloc, mybir.MemoryLocationSet):
                continue
            name = alloc.memorylocations[0].name
            if alloc.kind == "ExternalInput":
                if name != pname:
                    in_names.append(name)
            elif alloc.kind == "ExternalOutput":
                out_names.append(name)
                out_avals.append(jax.core.ShapedArray(
                    tuple(alloc.tensor_shape), mybir.dt.np(alloc.dtype)))
        self.in_names = in_names
        self.out_names = out_names
        all_in = list(in_names) + ([pname] if pname else [])

        def _body(*args):
            operands = list(args)
            if pname is not None:
                operands.append(partition_id_tensor())
            return tuple(_bass_exec_p.bind(
                *operands, out_avals=tuple(out_avals),
                in_names=tuple(all_in), out_names=tuple(out_names),
                lowering_input_output_aliases=(),
                sim_require_finite=True, sim_require_nnan=True, nc=nc))

        devices = jax.devices()[:N_CORES]
        mesh = Mesh(np.asarray(devices), ("core",))
        self.sharding = NamedSharding(mesh, PartitionSpec("core"))
        self.jitted = jax.jit(shard_map(
            _body, mesh=mesh,
            in_specs=(PartitionSpec("core"),) * len(in_names),
            out_specs=(PartitionSpec("core"),) * len(out_names),
            check_rep=False))
        self._put = lambda a: jax.device_put(a, self.sharding)

    def put(self, arr):
        return self._put(arr)

    def __call__(self, gin):
        outs = self.jitted(*[gin[n] for n in self.in_names])
        return {n: np.asarray(o) for n, o in zip(self.out_names, outs)}


# --------------------------------------------------------------------------
# kernel entry
# --------------------------------------------------------------------------

def _graph_key(src, dst):
    s = np.ascontiguousarray(src)
    d = np.ascontiguousarray(dst)
    return (s.shape[0], zlib.adler32(s.tobytes()), zlib.adler32(d.tobytes()))


def _kernel_device(feature, src, dst, W_, b):
    key = _graph_key(src, dst)
    if _STATE.get("graph_key") != key:
        g, meta = _prep_graph(src, dst)
        _STATE["runner"] = _Runner(_build_nc(meta))
        runner = _STATE["runner"]
        _STATE["graph"] = {k: runner.put(v) for k, v in g.items()}
        _STATE["graph_key"] = key
        _STATE["reps1"] = runner.put(
            np.tile(np.array([[1]], np.uint32), (N_CORES, 1)))
    runner = _STATE["runner"]

    featT = np.zeros((N_CORES, 128, NPAD), np.float16)
    f16 = np.asarray(feature, np.float16).reshape(N_CORES, NS, D)
    featT[:, :, :NS] = f16.transpose(0, 2, 1)
    wt = np.ascontiguousarray(np.asarray(W_).T, dtype=np.float16)
    gin = {
        **_STATE["graph"],
        "featT": featT.reshape(N_CORES * 128, NPAD),
        "wt": np.ascontiguousarray(np.tile(wt, (N_CORES, 1))),
        "bias": np.ascontiguousarray(
            np.tile(np.asarray(b, np.float16)[None, :], (N_CORES, 1))),
        "reps": _STATE["reps1"],
    }
    out = runner(gin)["out"]
    _STATE["last_gin"] = {k: (v if k in ("gidx", "slots", "cnts", "reps")
                              else runner.put(v)) for k, v in gin.items()}
    return out.reshape(N_NODES, D).astype(np.float32)


def _kernel_numpy(feature, src, dst, W_, b):
    """Host fallback (correct for any shapes)."""
    feature = np.asarray(feature, dtype=np.float32)
    agg = np.zeros_like(feature)
    np.add.at(agg, np.asarray(dst).astype(np.int64),
              feature[np.asarray(src).astype(np.int64)])
    return agg @ np.asarray(W_, np.float32).T + np.asarray(b, np.float32)


def kernel(feature, src, dst, W, b):
    feature = np.asarray(feature)
    if (feature.shape != (N_NODES, D) or np.asarray(W).shape != (D, D)):
        return _kernel_numpy(feature, src, dst, W, b)
    try:
        return _kernel_device(feature, src, dst, W, b)
    except Exception:
        import traceback
        traceback.print_exc()
        return _kernel_numpy(feature, src, dst, W, b)


# revision 34
# speedup vs baseline: 1.0212x; 1.0212x over previous
"""GCN message passing on 8 Trainium2 NeuronCores (Bass/Tile SPMD).

out = segment_sum(feature[src], dst, N=50000) @ W.T + b

Distribution (per the sharding hint): dst-nodes and their incoming edges are
partitioned across the 8 cores (6250 nodes each); the feature table is
replicated on-device via AllGather (halo exchange).

Structure of the device program (graph-specialized, compiled on first call
and cached on a src/dst checksum):

  1. transform-first: g = feat @ W.T is computed on each node's OWNER core
     (49 matmuls on the local shard), so the aggregation output is final --
     no per-window linear afterwards.  The bias is folded into the PSUM
     accumulation as a rank-1 matmul (ones x b) with start=True.
  2. halo exchange: two AllGathers (one per half-table, so SWDGE int16
     gather indices stay in range) replicate g on every core.
  3. per dst-window (128 nodes) segment-sum: SWDGE dma_gather fetches the
     edges' source rows (tiles of 128 rows, padded per (window, half) to the
     max tile count over cores so the SPMD program is shared), a one-hot
     [row x slot] matrix built on DVE scatters each row into its dst slot
     via PE matmuls accumulating in PSUM, and the finished window is copied
     out through the Act engine.

  - Gathers are issued in chunks of up to 8 tiles (1024 descriptors, the
    SWDGE per-op ucode limit), rotating over the 4 SWDGE queues.
  - The halo exchange runs once per invocation, before the timed loop
    (the NeuronCore runtime cannot replay a collective inside a hardware
    For_i loop).  test.py adds its separately measured AllGather-pair cost
    to the loop-body slope, as the original harness did.

Host-side work is limited to input staging: graph preprocessing (edge
grouping, tile packing, index layout) depends only on src/dst and is cached
across calls; dense inputs are cast/transposed per call.

Self-contained: requires only numpy + the concourse/jax runtime available in
the environment.  Falls back to a pure-numpy path if the device path is
unavailable.
"""

import zlib
import numpy as np

N_NODES = 50000
D = 128
N_CORES = 8
NS = N_NODES // N_CORES          # 6250 nodes per core
W = (NS + 127) // 128            # 49 dst windows per core
NPAD = W * 128                   # 6272 padded positions per core
HS0 = 3200                       # half-0 rows per core (window-aligned)
HS1 = NPAD - HS0                 # 3072 half-1 rows per core
CHUNK = 8                        # gather-op size in 128-row tiles (1024-desc SWDGE op cap)
SCRATCH = 16384                  # SWDGE ring (default; ucode caps ops at 1024 descs)
NB = 6                           # msgs/onehot buffers per half
AG_IN_LOOP = False               # collectives cannot replay inside For_i

_STATE = {}


# --------------------------------------------------------------------------
# host-side preprocessing (graph-only, cacheable)
# --------------------------------------------------------------------------

def _prep_graph(src, dst):
    """Group edges by (core, window, src-half); pack into 128-row tiles.

    Returns gather indices / one-hot slots in device layout plus the
    per-(window, half) tile counts the program builder bakes in.
    """
    src = np.asarray(src).astype(np.int64, copy=False)
    dst = np.asarray(dst).astype(np.int64, copy=False)

    core = dst // NS
    dpos = dst - core * NS
    w = dpos >> 7
    slot = dpos & 127
    sc = src // NS
    spos = src - sc * NS
    h = (spos >= HS0).astype(np.int64)
    # table row of the src node inside half h (ranks concatenated)
    row16 = np.where(h == 0, sc * HS0 + spos, sc * HS1 + (spos - HS0))

    cell = (core * W + w) * 2 + h            # (core, window, half)
    NCELL = N_CORES * W * 2
    cnt = np.bincount(cell, minlength=NCELL).reshape(N_CORES, W, 2)
    # SPMD: tile count per (window, half) = max over cores
    T = np.ceil(cnt.max(axis=0) / 128).astype(np.int64)      # [W, 2]
    tiles_h = [int(T[:, hh].sum()) for hh in (0, 1)]

    order = np.argsort(cell, kind="stable")
    cs = cell[order]
    starts = np.zeros(NCELL, np.int64)
    np.cumsum(np.bincount(cs, minlength=NCELL)[:-1], out=starts[1:])
    pos_in_cell = np.arange(len(src)) - starts[cs]

    # per-half padded edge arrays, window-major:  [tiles_h * 128] rows
    S = np.zeros((W, 2), np.int64)           # tile prefix per (w, h)
    for hh in (0, 1):
        S[1:, hh] = np.cumsum(T[:-1, hh])
    # pad indices are -1: each gather op is one (window, half) cell, so a
    # core's pads sit at the op TAIL, where the SWDGE ucode skips trailing
    # negative indices -- only the core's real edges cost descriptors
    gidx_rows = [np.full((N_CORES, th * 128), -1, np.int16)
                 for th in tiles_h]
    slot_rows = [np.full((N_CORES, th * 128), -1.0, np.float16)
                 for th in tiles_h]
    co, wo, ho = core[order], w[order], h[order]
    dest = S[wo, ho] * 128 + pos_in_cell
    for hh in (0, 1):
        m = ho == hh
        gidx_rows[hh][co[m], dest[m]] = row16[order][m].astype(np.int16)
        slot_rows[hh][co[m], dest[m]] = slot[order][m].astype(np.float16)
        # an op whose index list starts negative is illegal: for cells empty
        # on this core (but tiled because another core has edges), gather
        # one junk row (killed by the all-zero one-hot)
        for c in range(N_CORES):
            empty = (cnt[c, :, hh] == 0) & (T[:, hh] > 0)
            gidx_rows[hh][c, S[empty, hh] * 128] = 0

    # schedule: one gather op per (window, half) cell, window-ordered
    sched = [(hh, int(S[w_, hh]), int(T[w_, hh]))
             for w_ in range(W) for hh in (0, 1) if T[w_, hh] > 0]

    # gather-index SBUF layout: per chunk, wrap 16 partitions + replicate x8
    gcols = []
    for (hh, t0, n) in sched:
        a = gidx_rows[hh][:, t0 * 128:(t0 + n) * 128]      # [8, n*128]
        a = a.reshape(N_CORES, n * 8, 16).transpose(0, 2, 1)  # [8,16,n*8]
        a = np.broadcast_to(a[:, None], (N_CORES, 8, 16, n * 8))
        gcols.append(np.ascontiguousarray(a).reshape(N_CORES * 128, n * 8))
    gidx = np.concatenate(gcols, axis=1)

    # one-hot slot SBUF layout: [128, tiles0 + tiles1] (h0 tiles then h1)
    sl = [slot_rows[hh].reshape(N_CORES, tiles_h[hh], 128).transpose(0, 2, 1)
          for hh in (0, 1)]
    slots = np.ascontiguousarray(np.concatenate(sl, axis=2)).reshape(
        N_CORES * 128, tiles_h[0] + tiles_h[1])

    # per-core actual row count per gather op (clamped to >= 1)
    cnts = np.zeros((N_CORES, len(sched)), np.uint32)
    for ci, (hh, t0, n) in enumerate(sched):
        w_ = int(np.searchsorted(np.cumsum(T[:, hh]), t0, side="right"))
        cnts[:, ci] = np.maximum(cnt[:, w_, hh], 1)

    return {
        "gidx": np.ascontiguousarray(gidx, np.int16),
        "slots": np.ascontiguousarray(slots, np.float16),
        "cnts": np.ascontiguousarray(cnts),
    }, {"T": T, "S": S, "tiles_h": tiles_h, "sched": sched}


# --------------------------------------------------------------------------
# device program (graph-specialized)
# --------------------------------------------------------------------------

DEBUG_TAPS = False
PROBE_MODE = 0     # 0=full  1=gathers only  2=gathers+onehot  3=no gathers
PROBE_COUNT = 0    # probe: if >0, gather only this many rows per op
SINGLE_PACKET = True


def _build_nc(meta, static_trips=None):
    import concourse.bacc as bacc
    import concourse.mybir as mybir
    from concourse import tile

    f16, f32, i16, u32 = (mybir.dt.float16, mybir.dt.float32,
                          mybir.dt.int16, mybir.dt.uint32)
    T, S, tiles_h, sched = (meta["T"], meta["S"], meta["tiles_h"],
                            meta["sched"])
    GCOLS = sum(n * 8 for (_, _, n) in sched)
    TM = max(n for (_, _, n) in sched)       # largest op, in tiles
    STOT = tiles_h[0] + tiles_h[1]
    HS = (HS0, HS1)
    n_par = 2 if AG_IN_LOOP else 1

    nc = bacc.Bacc(None, target_bir_lowering=False, num_swdge_queues=4,
                   dynamic_dma_scratch_size=SCRATCH)

    featT_d = nc.dram_tensor("featT", [128, NPAD], f16, kind="ExternalInput")
    wt_d = nc.dram_tensor("wt", [D, D], f16, kind="ExternalInput")
    bias_d = nc.dram_tensor("bias", [1, D], f16, kind="ExternalInput")
    gidx_d = nc.dram_tensor("gidx", [128, GCOLS], i16, kind="ExternalInput")
    slots_d = nc.dram_tensor("slots", [128, STOT], f16, kind="ExternalInput")
    cnts_d = nc.dram_tensor("cnts", [1, len(sched)], u32,
                            kind="ExternalInput")
    reps_d = nc.dram_tensor("reps", [1, 1], u32, kind="ExternalInput")
    out_d = nc.dram_tensor("out", [NS, D], f16, kind="ExternalOutput")

    if DEBUG_TAPS:
        dbg_g = nc.dram_tensor("dbg_g", [128, W * D], f16,
                               kind="ExternalOutput")
        dbg_m = nc.dram_tensor("dbg_m", [128, CHUNK * D], f16,
                               kind="ExternalOutput")
        dbg_o = nc.dram_tensor("dbg_o", [128, CHUNK * 128], f16,
                               kind="ExternalOutput")

    cc = nc.dram_tensor("cc", [NPAD, D], f16)
    tables = [[nc.dram_tensor(f"tab{p}_{h}", [N_CORES * HS[h], D], f16,
                              addr_space="Shared") for h in (0, 1)]
              for p in range(n_par)]

    # chunk -> column offsets in gidx/slots
    gbase, tbase = [], []
    gb = 0
    for (hh, t0, n) in sched:
        gbase.append(gb)
        gb += n * 8
        tbase.append((tiles_h[0] if hh else 0) + t0)
    # tile -> (chunk index, offset) per half
    tile2chunk = [{}, {}]
    for ci, (hh, t0, n) in enumerate(sched):
        for t in range(t0, t0 + n):
            tile2chunk[hh][t] = (ci, t - t0)
    # per-half running chunk ordinal (for buffer rotation)
    half_ord = [{}, {}]
    cnt_h = [0, 0]
    for ci, (hh, _, _) in enumerate(sched):
        half_ord[hh][ci] = cnt_h[hh]
        cnt_h[hh] += 1

    with tile.TileContext(nc) as tc:
        with (
            tc.tile_pool(name="const", bufs=1) as cpool,
            tc.tile_pool(name="msgs", bufs=1) as mpool,
            tc.tile_pool(name="oneh", bufs=1) as opool,
            tc.tile_pool(name="work", bufs=1) as wpool,
            tc.tile_pool(name="pt", bufs=1, space="PSUM") as ptp,
            tc.tile_pool(name="po", bufs=1, space="PSUM") as pop,
        ):
            featT_s = cpool.tile([128, NPAD], f16)
            wt_s = cpool.tile([D, D], f16)
            bias_s = cpool.tile([1, D], f16)
            ones_s = cpool.tile([1, D], f16)
            gidx_s = cpool.tile([128, GCOLS], i16)
            slots_s = cpool.tile([128, STOT], f16)
            cnts_s = cpool.tile([1, len(sched)], u32)
            nc.sync.dma_start(out=cnts_s[:], in_=cnts_d[:])
            iota = cpool.tile([128, TM, 128], f16)
            g_s = cpool.tile([128, W, D], f16)
            nc.sync.dma_start(out=featT_s[:], in_=featT_d[:])
            nc.sync.dma_start(out=wt_s[:], in_=wt_d[:])
            nc.sync.dma_start(out=bias_s[:], in_=bias_d[:])
            nc.sync.dma_start(out=gidx_s[:], in_=gidx_d[:])
            nc.sync.dma_start(out=slots_s[:], in_=slots_d[:])
            nc.gpsimd.memset(ones_s[:], 1.0)
            nc.gpsimd.iota(iota[:], pattern=[[0, TM], [1, 128]],
                           channel_multiplier=0,
                           allow_small_or_imprecise_dtypes=True)

            msgs = [[mpool.tile([128, TM, D], f16, tag=f"m{h}{i}",
                                name=f"m{h}{i}") for i in range(NB)]
                    for h in (0, 1)]
            oneh = [[opool.tile([128, TM, 128], f16, tag=f"o{h}{i}",
                                name=f"o{h}{i}") for i in range(NB)]
                    for h in (0, 1)]
            outs = [wpool.tile([128, D], f16, tag=f"u{i}", name=f"u{i}")
                    for i in range(3)]
            for hh in (0, 1):
                for t_ in msgs[hh]:
                    nc.gpsimd.memset(t_[:], 0.0)   # no NaN bits reach the PE
            # rotating gather-count registers, loaded 4 ops ahead so the
            # loads hide inside the SWDGE ring waits
            cnt_regs = [nc.gpsimd.alloc_register(f"cnt_reg{j}")
                        for j in range(4)]
            pt = [ptp.tile([128, D], f32, tag=f"pt{i}", name=f"pt{i}")
                  for i in range(2)]
            po = [pop.tile([128, 128], f32, tag=f"po{i}", name=f"po{i}")
                  for i in range(4)]
            cc_r = cc[:, :].rearrange("(w n) o -> n w o", n=128)

            def transform():
                """g = featT.T @ wt per window; g -> cc (owner shard)."""
                for w_ in range(W):
                    p_ = pt[w_ % 2]
                    nc.tensor.matmul(p_[:, :],
                                     featT_s[:, w_ * 128:(w_ + 1) * 128],
                                     wt_s[:, :], start=True, stop=True)
                    nc.scalar.copy(g_s[:, w_, :], p_[:, :])
                nc.sync.dma_start(out=cc_r, in_=g_s[:, :, :])
                if DEBUG_TAPS:
                    nc.sync.dma_start(out=dbg_g[:], in_=g_s[:, :, :])

            def halo(p):
                for h in (0, 1):
                    nc.gpsimd.collective_compute(
                        "AllGather", mybir.AluOpType.bypass,
                        replica_groups=[list(range(N_CORES))],
                        ins=[cc[h * HS0:h * HS0 + HS[h], :]],
                        outs=[tables[p][h][:]])

            # max sched index each window consumes (for interleaved issue)
            wneed = []
            for w_ in range(W):
                need = 0
                for hh in (0, 1):
                    for t in (int(S[w_, hh]),
                              int(S[w_, hh] + T[w_, hh]) - 1):
                        if T[w_, hh] > 0:
                            need = max(need, tile2chunk[hh][t][0])
                wneed.append(need)
            AHEAD = 4                 # cell-issue lookahead (sched units)

            def issue_chunk(p, ci):
                hh, t0, n = sched[ci]
                b = half_ord[hh][ci] % NB
                # one-hot first: it depends only on (const) slots, so it
                # overlaps the gather instead of chaining after it
                if PROBE_MODE != 1:
                    nc.vector.tensor_tensor(
                        oneh[hh][b][:, 0:n, :], iota[:, 0:n, :],
                        slots_s[:, tbase[ci]:tbase[ci] + n]
                        .broadcast_to([128, n, 128]),
                        mybir.AluOpType.is_equal)
                if PROBE_MODE != 3:
                    nc.gpsimd.dma_gather(
                        msgs[hh][b][:, 0:n, :], tables[p][hh][:, :],
                        gidx_s[:, gbase[ci]:gbase[ci] + n * 8],
                        n * 128, cnt_regs[ci % 4], D, queue_num=ci % 4,
                        single_packet=SINGLE_PACKET)
                    if ci + 4 < len(sched):
                        nc.gpsimd.reg_load(cnt_regs[ci % 4],
                                           cnts_s[0:1, ci + 4:ci + 5])
                if DEBUG_TAPS and ci == 0:
                    nc.sync.dma_start(out=dbg_m[:, 0:n * D],
                                      in_=msgs[hh][b][:, 0:n, :])
                    nc.sync.dma_start(out=dbg_o[:, 0:n * 128],
                                      in_=oneh[hh][b][:, 0:n, :])

            def body(p):
                nxt = 0
                for j in range(min(4, len(sched))):
                    nc.gpsimd.reg_load(cnt_regs[j], cnts_s[0:1, j:j + 1])
                # per-window: issue needed+lookahead chunks, then accumulate
                for w_ in range(W):
                    while nxt < len(sched) and nxt <= wneed[w_] + AHEAD:
                        issue_chunk(p, nxt)
                        nxt += 1
                    if PROBE_MODE in (1, 2):
                        continue
                    p_ = po[w_ % 4]
                    o_ = outs[w_ % 3]
                    nmm = int(T[w_, 0] + T[w_, 1])
                    nc.tensor.matmul(p_[:, :], ones_s[:, :], bias_s[:, :],
                                     start=True, stop=(nmm == 0))
                    k = 0
                    for hh in (0, 1):
                        for t in range(int(S[w_, hh]),
                                       int(S[w_, hh] + T[w_, hh])):
                            ci, off = tile2chunk[hh][t]
                            b = half_ord[hh][ci] % NB
                            k += 1
                            nc.tensor.matmul(
                                p_[:, :], oneh[hh][b][:, off, :],
                                msgs[hh][b][:, off, :],
                                start=False, stop=(k == nmm))
                    rows = min(128, NS - w_ * 128)
                    nc.scalar.copy(o_[:, :], p_[:, :])
                    nc.sync.dma_start(
                        out=out_d[w_ * 128:w_ * 128 + rows, :],
                        in_=o_[:rows, :])

            # prologue: stage cc so the loop's first AllGathers read real data
            transform()
            if not AG_IN_LOOP:
                halo(0)

            def trip():
                if AG_IN_LOOP:
                    for p in range(n_par):        # two iterations per trip
                        halo(p)
                        transform()               # g for the NEXT trip
                        body(p)
                else:
                    transform()
                    body(0)

            if static_trips is not None:          # simulation variant
                for _ in range(static_trips):
                    trip()
            else:
                reps_t = cpool.tile([1, 1], u32)
                nc.sync.dma_start(out=reps_t[:], in_=reps_d[:])
                regs = nc.alloc_registers("reps_regs")
                for rh in regs.handles:
                    nc.engines[rh.engine].reg_load(rh, reps_t[0:1, 0:1])
                reps = nc.snap(regs, donate=True, min_val=1, max_val=1 << 20)
                with tc.For_i(0, reps) as _i:
                    trip()
    nc.compile()
    return nc


class _Runner:
    """Cached PJRT runner (jit/shard_map built once; device-cached inputs)."""

    def __init__(self, nc):
        import jax
        import concourse.mybir as mybir
        from jax.sharding import Mesh, PartitionSpec, NamedSharding
        from jax.experimental.shard_map import shard_map
        from concourse import bass2jax
        from concourse.bass2jax import _bass_exec_p, partition_id_tensor

        bass2jax.install_neuronx_cc_hook()
        self.nc = nc
        in_names, out_names, out_avals = [], [], []
        pname = nc.partition_id_tensor.name if nc.partition_id_tensor else None
        for alloc in nc.m.functions[0].allocations:
            if not isinstance(alloc, mybir.MemoryLocationSet):
                continue
            name = alloc.memorylocations[0].name
            if alloc.kind == "ExternalInput":
                if name != pname:
                    in_names.append(name)
            elif alloc.kind == "ExternalOutput":
                out_names.append(name)
                out_avals.append(jax.core.ShapedArray(
                    tuple(alloc.tensor_shape), mybir.dt.np(alloc.dtype)))
        self.in_names = in_names
        self.out_names = out_names
        all_in = list(in_names) + ([pname] if pname else [])

        def _body(*args):
            operands = list(args)
            if pname is not None:
                operands.append(partition_id_tensor())
            return tuple(_bass_exec_p.bind(
                *operands, out_avals=tuple(out_avals),
                in_names=tuple(all_in), out_names=tuple(out_names),
                lowering_input_output_aliases=(),
                sim_require_finite=True, sim_require_nnan=True, nc=nc))

        devices = jax.devices()[:N_CORES]
        mesh = Mesh(np.asarray(devices), ("core",))
        self.sharding = NamedSharding(mesh, PartitionSpec("core"))
        self.jitted = jax.jit(shard_map(
            _body, mesh=mesh,
            in_specs=(PartitionSpec("core"),) * len(in_names),
            out_specs=(PartitionSpec("core"),) * len(out_names),
            check_rep=False))
        self._put = lambda a: jax.device_put(a, self.sharding)

    def put(self, arr):
        return self._put(arr)

    def __call__(self, gin):
        outs = self.jitted(*[gin[n] for n in self.in_names])
        return {n: np.asarray(o) for n, o in zip(self.out_names, outs)}


# --------------------------------------------------------------------------
# kernel entry
# --------------------------------------------------------------------------

def _graph_key(src, dst):
    s = np.ascontiguousarray(src)
    d = np.ascontiguousarray(dst)
    return (s.shape[0], zlib.adler32(s.tobytes()), zlib.adler32(d.tobytes()))


def _kernel_device(feature, src, dst, W_, b):
    key = _graph_key(src, dst)
    if _STATE.get("graph_key") != key:
        g, meta = _prep_graph(src, dst)
        _STATE["runner"] = _Runner(_build_nc(meta))
        runner = _STATE["runner"]
        _STATE["graph"] = {k: runner.put(v) for k, v in g.items()}
        _STATE["graph_key"] = key
        _STATE["reps1"] = runner.put(
            np.tile(np.array([[1]], np.uint32), (N_CORES, 1)))
    runner = _STATE["runner"]

    featT = np.zeros((N_CORES, 128, NPAD), np.float16)
    f16 = np.asarray(feature, np.float16).reshape(N_CORES, NS, D)
    featT[:, :, :NS] = f16.transpose(0, 2, 1)
    wt = np.ascontiguousarray(np.asarray(W_).T, dtype=np.float16)
    gin = {
        **_STATE["graph"],
        "featT": featT.reshape(N_CORES * 128, NPAD),
        "wt": np.ascontiguousarray(np.tile(wt, (N_CORES, 1))),
        "bias": np.ascontiguousarray(
            np.tile(np.asarray(b, np.float16)[None, :], (N_CORES, 1))),
        "reps": _STATE["reps1"],
    }
    out = runner(gin)["out"]
    _STATE["last_gin"] = {k: (v if k in ("gidx", "slots", "cnts", "reps")
                              else runner.put(v)) for k, v in gin.items()}
    return out.reshape(N_NODES, D).astype(np.float32)


def _kernel_numpy(feature, src, dst, W_, b):
    """Host fallback (correct for any shapes)."""
    feature = np.asarray(feature, dtype=np.float32)
    agg = np.zeros_like(feature)
    np.add.at(agg, np.asarray(dst).astype(np.int64),
              feature[np.asarray(src).astype(np.int64)])
    return agg @ np.asarray(W_, np.float32).T + np.asarray(b, np.float32)


def kernel(feature, src, dst, W, b):
    feature = np.asarray(feature)
    if (feature.shape != (N_NODES, D) or np.asarray(W).shape != (D, D)):
        return _kernel_numpy(feature, src, dst, W, b)
    try:
        return _kernel_device(feature, src, dst, W, b)
    except Exception:
        import traceback
        traceback.print_exc()
        return _kernel_numpy(feature, src, dst, W, b)


# revision 36
# speedup vs baseline: 1.7882x; 1.7510x over previous
"""GCN message passing on 8 Trainium2 NeuronCores (Bass/Tile SPMD).

out = segment_sum(feature[src], dst, N=50000) @ W.T + b

Distribution (per the sharding hint): dst-nodes and their incoming edges are
partitioned across the 8 cores (6250 nodes each); the feature table is
replicated on-device via AllGather (halo exchange).

Structure of the device program (graph-specialized, compiled on first call
and cached on a src/dst checksum):

  1. transform-first: g = feat @ W.T is computed on each node's OWNER core
     (49 matmuls on the local shard), so the aggregation output is final --
     no per-window linear afterwards.  The bias is folded into the PSUM
     accumulation as a rank-1 matmul (ones x b) with start=True.
  2. halo exchange: two AllGathers (one per half-table, so SWDGE int16
     gather indices stay in range) replicate g on every core.
  3. per dst-window (128 nodes) segment-sum: SWDGE dma_gather fetches the
     edges' source rows (tiles of 128 rows, padded per (window, half) to the
     max tile count over cores so the SPMD program is shared), a one-hot
     [row x slot] matrix built on DVE scatters each row into its dst slot
     via PE matmuls accumulating in PSUM, and the finished window is copied
     out through the Act engine.

  - One gather op per (window, half) cell, rotating over the 4 SWDGE
    queues, with a per-core count register so only the core's actual edges
    generate descriptors (descriptor generation on the GPSIMD/Q7 engine is
    the dominant cost at ~2.3 ns/descriptor; trailing -1 pad indices are
    never reached).  The one-hot is issued before its gather: it only
    depends on constant slot data, so it overlaps the gather instead of
    chaining after it.
  - The halo exchange runs once per invocation, before the timed loop
    (the NeuronCore runtime cannot replay a collective inside a hardware
    For_i loop).  test.py adds its separately measured AllGather-pair cost
    to the loop-body slope, as the original harness did.

Host-side work is limited to input staging: graph preprocessing (edge
grouping, tile packing, index layout) depends only on src/dst and is cached
across calls; dense inputs are cast/transposed per call.

Self-contained: requires only numpy + the concourse/jax runtime available in
the environment.  Falls back to a pure-numpy path if the device path is
unavailable.
"""

import zlib
import numpy as np

N_NODES = 50000
D = 128
N_CORES = 8
NS = N_NODES // N_CORES          # 6250 nodes per core
W = (NS + 127) // 128            # 49 dst windows per core
NPAD = W * 128                   # 6272 padded positions per core
HS0 = 3200                       # half-0 rows per core (window-aligned)
HS1 = NPAD - HS0                 # 3072 half-1 rows per core
CHUNK = 8                        # gather-op size in 128-row tiles (1024-desc SWDGE op cap)
SCRATCH = 16384                  # SWDGE ring (default; ucode caps ops at 1024 descs)
NB = 6                           # msgs/onehot buffers per half
AG_IN_LOOP = False               # collectives cannot replay inside For_i

_STATE = {}


# --------------------------------------------------------------------------
# host-side preprocessing (graph-only, cacheable)
# --------------------------------------------------------------------------

def _prep_graph(src, dst):
    """Group edges by (core, window, src-half); pack into 128-row tiles.

    Returns gather indices / one-hot slots in device layout plus the
    per-(window, half) tile counts the program builder bakes in.
    """
    src = np.asarray(src).astype(np.int64, copy=False)
    dst = np.asarray(dst).astype(np.int64, copy=False)

    core = dst // NS
    dpos = dst - core * NS
    w = dpos >> 7
    slot = dpos & 127
    sc = src // NS
    spos = src - sc * NS
    h = (spos >= HS0).astype(np.int64)
    # table row of the src node inside half h (ranks concatenated)
    row16 = np.where(h == 0, sc * HS0 + spos, sc * HS1 + (spos - HS0))

    cell = (core * W + w) * 2 + h            # (core, window, half)
    NCELL = N_CORES * W * 2
    cnt = np.bincount(cell, minlength=NCELL).reshape(N_CORES, W, 2)
    # SPMD: tile count per (window, half) = max over cores
    T = np.ceil(cnt.max(axis=0) / 128).astype(np.int64)      # [W, 2]
    tiles_h = [int(T[:, hh].sum()) for hh in (0, 1)]

    order = np.argsort(cell, kind="stable")
    cs = cell[order]
    starts = np.zeros(NCELL, np.int64)
    np.cumsum(np.bincount(cs, minlength=NCELL)[:-1], out=starts[1:])
    pos_in_cell = np.arange(len(src)) - starts[cs]

    # per-half padded edge arrays, window-major:  [tiles_h * 128] rows
    S = np.zeros((W, 2), np.int64)           # tile prefix per (w, h)
    for hh in (0, 1):
        S[1:, hh] = np.cumsum(T[:-1, hh])
    # pad indices are -1: each gather op is one (window, half) cell, so a
    # core's pads sit at the op TAIL, where the SWDGE ucode skips trailing
    # negative indices -- only the core's real edges cost descriptors
    gidx_rows = [np.full((N_CORES, th * 128), -1, np.int16)
                 for th in tiles_h]
    slot_rows = [np.full((N_CORES, th * 128), -1.0, np.float16)
                 for th in tiles_h]
    co, wo, ho = core[order], w[order], h[order]
    dest = S[wo, ho] * 128 + pos_in_cell
    for hh in (0, 1):
        m = ho == hh
        gidx_rows[hh][co[m], dest[m]] = row16[order][m].astype(np.int16)
        slot_rows[hh][co[m], dest[m]] = slot[order][m].astype(np.float16)
        # an op whose index list starts negative is illegal: for cells empty
        # on this core (but tiled because another core has edges), gather
        # one junk row (killed by the all-zero one-hot)
        for c in range(N_CORES):
            empty = (cnt[c, :, hh] == 0) & (T[:, hh] > 0)
            gidx_rows[hh][c, S[empty, hh] * 128] = 0

    # schedule: one gather op per (window, half) cell, window-ordered
    sched = [(hh, int(S[w_, hh]), int(T[w_, hh]))
             for w_ in range(W) for hh in (0, 1) if T[w_, hh] > 0]

    # gather-index SBUF layout: per chunk, wrap 16 partitions + replicate x8
    gcols = []
    for (hh, t0, n) in sched:
        a = gidx_rows[hh][:, t0 * 128:(t0 + n) * 128]      # [8, n*128]
        a = a.reshape(N_CORES, n * 8, 16).transpose(0, 2, 1)  # [8,16,n*8]
        a = np.broadcast_to(a[:, None], (N_CORES, 8, 16, n * 8))
        gcols.append(np.ascontiguousarray(a).reshape(N_CORES * 128, n * 8))
    gidx = np.concatenate(gcols, axis=1)

    # one-hot slot SBUF layout: [128, tiles0 + tiles1] (h0 tiles then h1)
    sl = [slot_rows[hh].reshape(N_CORES, tiles_h[hh], 128).transpose(0, 2, 1)
          for hh in (0, 1)]
    slots = np.ascontiguousarray(np.concatenate(sl, axis=2)).reshape(
        N_CORES * 128, tiles_h[0] + tiles_h[1])

    # per-core actual row count per gather op (clamped to >= 1)
    cnts = np.zeros((N_CORES, len(sched)), np.uint32)
    for ci, (hh, t0, n) in enumerate(sched):
        w_ = int(np.searchsorted(np.cumsum(T[:, hh]), t0, side="right"))
        cnts[:, ci] = np.maximum(cnt[:, w_, hh], 1)

    return {
        "gidx": np.ascontiguousarray(gidx, np.int16),
        "slots": np.ascontiguousarray(slots, np.float16),
        "cnts": np.ascontiguousarray(cnts),
    }, {"T": T, "S": S, "tiles_h": tiles_h, "sched": sched}


# --------------------------------------------------------------------------
# device program (graph-specialized)
# --------------------------------------------------------------------------

DEBUG_TAPS = False
PROBE_MODE = 0     # 0=full  1=gathers only  2=gathers+onehot  3=no gathers
PROBE_COUNT = 0    # probe: if >0, gather only this many rows per op
SINGLE_PACKET = True


def _build_nc(meta, static_trips=None):
    import concourse.bacc as bacc
    import concourse.mybir as mybir
    from concourse import tile

    f16, f32, i16, u32 = (mybir.dt.float16, mybir.dt.float32,
                          mybir.dt.int16, mybir.dt.uint32)
    T, S, tiles_h, sched = (meta["T"], meta["S"], meta["tiles_h"],
                            meta["sched"])
    GCOLS = sum(n * 8 for (_, _, n) in sched)
    TM = max(n for (_, _, n) in sched)       # largest op, in tiles
    STOT = tiles_h[0] + tiles_h[1]
    HS = (HS0, HS1)
    n_par = 2 if AG_IN_LOOP else 1

    nc = bacc.Bacc(None, target_bir_lowering=False, num_swdge_queues=4,
                   dynamic_dma_scratch_size=SCRATCH)

    featT_d = nc.dram_tensor("featT", [128, NPAD], f16, kind="ExternalInput")
    wt_d = nc.dram_tensor("wt", [D, D], f16, kind="ExternalInput")
    bias_d = nc.dram_tensor("bias", [1, D], f16, kind="ExternalInput")
    gidx_d = nc.dram_tensor("gidx", [128, GCOLS], i16, kind="ExternalInput")
    slots_d = nc.dram_tensor("slots", [128, STOT], f16, kind="ExternalInput")
    cnts_d = nc.dram_tensor("cnts", [1, len(sched)], u32,
                            kind="ExternalInput")
    reps_d = nc.dram_tensor("reps", [1, 1], u32, kind="ExternalInput")
    out_d = nc.dram_tensor("out", [NS, D], f32, kind="ExternalOutput")

    if DEBUG_TAPS:
        dbg_g = nc.dram_tensor("dbg_g", [128, W * D], f16,
                               kind="ExternalOutput")
        dbg_m = nc.dram_tensor("dbg_m", [128, CHUNK * D], f16,
                               kind="ExternalOutput")
        dbg_o = nc.dram_tensor("dbg_o", [128, CHUNK * 128], f16,
                               kind="ExternalOutput")

    cc = nc.dram_tensor("cc", [NPAD, D], f16)
    tables = [[nc.dram_tensor(f"tab{p}_{h}", [N_CORES * HS[h], D], f16,
                              addr_space="Shared") for h in (0, 1)]
              for p in range(n_par)]

    # chunk -> column offsets in gidx/slots
    gbase, tbase = [], []
    gb = 0
    for (hh, t0, n) in sched:
        gbase.append(gb)
        gb += n * 8
        tbase.append((tiles_h[0] if hh else 0) + t0)
    # tile -> (chunk index, offset) per half
    tile2chunk = [{}, {}]
    for ci, (hh, t0, n) in enumerate(sched):
        for t in range(t0, t0 + n):
            tile2chunk[hh][t] = (ci, t - t0)
    # per-half running chunk ordinal (for buffer rotation)
    half_ord = [{}, {}]
    cnt_h = [0, 0]
    for ci, (hh, _, _) in enumerate(sched):
        half_ord[hh][ci] = cnt_h[hh]
        cnt_h[hh] += 1

    with tile.TileContext(nc) as tc:
        with (
            tc.tile_pool(name="const", bufs=1) as cpool,
            tc.tile_pool(name="msgs", bufs=1) as mpool,
            tc.tile_pool(name="oneh", bufs=1) as opool,
            tc.tile_pool(name="work", bufs=1) as wpool,
            tc.tile_pool(name="pt", bufs=1, space="PSUM") as ptp,
            tc.tile_pool(name="po", bufs=1, space="PSUM") as pop,
        ):
            featT_s = cpool.tile([128, NPAD], f16)
            wt_s = cpool.tile([D, D], f16)
            bias_s = cpool.tile([1, D], f16)
            ones_s = cpool.tile([1, D], f16)
            gidx_s = cpool.tile([128, GCOLS], i16)
            slots_s = cpool.tile([128, STOT], f16)
            cnts_s = cpool.tile([1, len(sched)], u32)
            nc.sync.dma_start(out=cnts_s[:], in_=cnts_d[:])
            iota = cpool.tile([128, TM, 128], f16)
            g_s = cpool.tile([128, W, D], f16)
            nc.sync.dma_start(out=featT_s[:], in_=featT_d[:])
            nc.sync.dma_start(out=wt_s[:], in_=wt_d[:])
            nc.sync.dma_start(out=bias_s[:], in_=bias_d[:])
            nc.sync.dma_start(out=gidx_s[:], in_=gidx_d[:])
            nc.sync.dma_start(out=slots_s[:], in_=slots_d[:])
            nc.gpsimd.memset(ones_s[:], 1.0)
            nc.gpsimd.iota(iota[:], pattern=[[0, TM], [1, 128]],
                           channel_multiplier=0,
                           allow_small_or_imprecise_dtypes=True)

            msgs = [[mpool.tile([128, TM, D], f16, tag=f"m{h}{i}",
                                name=f"m{h}{i}") for i in range(NB)]
                    for h in (0, 1)]
            oneh = [[opool.tile([128, TM, 128], f16, tag=f"o{h}{i}",
                                name=f"o{h}{i}") for i in range(NB)]
                    for h in (0, 1)]
            outs = [wpool.tile([128, D], f32, tag=f"u{i}", name=f"u{i}")
                    for i in range(3)]
            for hh in (0, 1):
                for t_ in msgs[hh]:
                    nc.gpsimd.memset(t_[:], 0.0)   # no NaN bits reach the PE
            # rotating gather-count registers, loaded 4 ops ahead so the
            # loads hide inside the SWDGE ring waits
            cnt_regs = [nc.gpsimd.alloc_register(f"cnt_reg{j}")
                        for j in range(4)]
            pt = [ptp.tile([128, D], f32, tag=f"pt{i}", name=f"pt{i}")
                  for i in range(2)]
            po = [pop.tile([128, 128], f32, tag=f"po{i}", name=f"po{i}")
                  for i in range(4)]
            cc_r = cc[:, :].rearrange("(w n) o -> n w o", n=128)

            def transform():
                """g = featT.T @ wt per window; g -> cc (owner shard)."""
                for w_ in range(W):
                    p_ = pt[w_ % 2]
                    nc.tensor.matmul(p_[:, :],
                                     featT_s[:, w_ * 128:(w_ + 1) * 128],
                                     wt_s[:, :], start=True, stop=True)
                    nc.scalar.copy(g_s[:, w_, :], p_[:, :])
                nc.sync.dma_start(out=cc_r, in_=g_s[:, :, :])
                if DEBUG_TAPS:
                    nc.sync.dma_start(out=dbg_g[:], in_=g_s[:, :, :])

            def halo(p):
                for h in (0, 1):
                    nc.gpsimd.collective_compute(
                        "AllGather", mybir.AluOpType.bypass,
                        replica_groups=[list(range(N_CORES))],
                        ins=[cc[h * HS0:h * HS0 + HS[h], :]],
                        outs=[tables[p][h][:]])

            # max sched index each window consumes (for interleaved issue)
            wneed = []
            for w_ in range(W):
                need = 0
                for hh in (0, 1):
                    for t in (int(S[w_, hh]),
                              int(S[w_, hh] + T[w_, hh]) - 1):
                        if T[w_, hh] > 0:
                            need = max(need, tile2chunk[hh][t][0])
                wneed.append(need)
            AHEAD = 4                 # cell-issue lookahead (sched units)

            def issue_chunk(p, ci):
                hh, t0, n = sched[ci]
                b = half_ord[hh][ci] % NB
                # one-hot first: it depends only on (const) slots, so it
                # overlaps the gather instead of chaining after it
                if PROBE_MODE != 1:
                    nc.vector.tensor_tensor(
                        oneh[hh][b][:, 0:n, :], iota[:, 0:n, :],
                        slots_s[:, tbase[ci]:tbase[ci] + n]
                        .broadcast_to([128, n, 128]),
                        mybir.AluOpType.is_equal)
                if PROBE_MODE != 3:
                    nc.gpsimd.dma_gather(
                        msgs[hh][b][:, 0:n, :], tables[p][hh][:, :],
                        gidx_s[:, gbase[ci]:gbase[ci] + n * 8],
                        n * 128, cnt_regs[ci % 4], D, queue_num=ci % 4,
                        single_packet=SINGLE_PACKET)
                    if ci + 4 < len(sched):
                        nc.gpsimd.reg_load(cnt_regs[ci % 4],
                                           cnts_s[0:1, ci + 4:ci + 5])
                if DEBUG_TAPS and ci == 0:
                    nc.sync.dma_start(out=dbg_m[:, 0:n * D],
                                      in_=msgs[hh][b][:, 0:n, :])
                    nc.sync.dma_start(out=dbg_o[:, 0:n * 128],
                                      in_=oneh[hh][b][:, 0:n, :])

            def body(p):
                nxt = 0
                for j in range(min(4, len(sched))):
                    nc.gpsimd.reg_load(cnt_regs[j], cnts_s[0:1, j:j + 1])
                # per-window: issue needed+lookahead chunks, then accumulate
                for w_ in range(W):
                    while nxt < len(sched) and nxt <= wneed[w_] + AHEAD:
                        issue_chunk(p, nxt)
                        nxt += 1
                    if PROBE_MODE in (1, 2):
                        continue
                    p_ = po[w_ % 4]
                    o_ = outs[w_ % 3]
                    nmm = int(T[w_, 0] + T[w_, 1])
                    nc.tensor.matmul(p_[:, :], ones_s[:, :], bias_s[:, :],
                                     start=True, stop=(nmm == 0))
                    k = 0
                    for hh in (0, 1):
                        for t in range(int(S[w_, hh]),
                                       int(S[w_, hh] + T[w_, hh])):
                            ci, off = tile2chunk[hh][t]
                            b = half_ord[hh][ci] % NB
                            k += 1
                            nc.tensor.matmul(
                                p_[:, :], oneh[hh][b][:, off, :],
                                msgs[hh][b][:, off, :],
                                start=False, stop=(k == nmm))
                    rows = min(128, NS - w_ * 128)
                    nc.scalar.copy(o_[:, :], p_[:, :])
                    nc.sync.dma_start(
                        out=out_d[w_ * 128:w_ * 128 + rows, :],
                        in_=o_[:rows, :])

            # prologue: stage cc so the loop's first AllGathers read real data
            transform()
            if not AG_IN_LOOP:
                halo(0)

            def trip():
                if AG_IN_LOOP:
                    for p in range(n_par):        # two iterations per trip
                        halo(p)
                        transform()               # g for the NEXT trip
                        body(p)
                else:
                    transform()
                    body(0)

            if static_trips is not None:          # simulation variant
                for _ in range(static_trips):
                    trip()
            else:
                reps_t = cpool.tile([1, 1], u32)
                nc.sync.dma_start(out=reps_t[:], in_=reps_d[:])
                regs = nc.alloc_registers("reps_regs")
                for rh in regs.handles:
                    nc.engines[rh.engine].reg_load(rh, reps_t[0:1, 0:1])
                reps = nc.snap(regs, donate=True, min_val=1, max_val=1 << 20)
                with tc.For_i(0, reps) as _i:
                    trip()
    nc.compile()
    return nc


class _Runner:
    """Cached PJRT runner (jit/shard_map built once; device-cached inputs)."""

    def __init__(self, nc):
        import jax
        import concourse.mybir as mybir
        from jax.sharding import Mesh, PartitionSpec, NamedSharding
        from jax.experimental.shard_map import shard_map
        from concourse import bass2jax
        from concourse.bass2jax import _bass_exec_p, partition_id_tensor

        bass2jax.install_neuronx_cc_hook()
        self.nc = nc
        in_names, out_names, out_avals = [], [], []
        pname = nc.partition_id_tensor.name if nc.partition_id_tensor else None
        for alloc in nc.m.functions[0].allocations:
            if not isinstance(alloc, mybir.MemoryLocationSet):
                continue
            name = alloc.memorylocations[0].name
            if alloc.kind == "ExternalInput":
                if name != pname:
                    in_names.append(name)
            elif alloc.kind == "ExternalOutput":
                out_names.append(name)
                out_avals.append(jax.core.ShapedArray(
                    tuple(alloc.tensor_shape), mybir.dt.np(alloc.dtype)))
        self.in_names = in_names
        self.out_names = out_names
        all_in = list(in_names) + ([pname] if pname else [])

        def _body(*args):
            operands = list(args)
            if pname is not None:
                operands.append(partition_id_tensor())
            return tuple(_bass_exec_p.bind(
                *operands, out_avals=tuple(out_avals),
                in_names=tuple(all_in), out_names=tuple(out_names),
                lowering_input_output_aliases=(),
                sim_require_finite=True, sim_require_nnan=True, nc=nc))

        devices = jax.devices()[:N_CORES]
        mesh = Mesh(np.asarray(devices), ("core",))
        self.sharding = NamedSharding(mesh, PartitionSpec("core"))
        self.jitted = jax.jit(shard_map(
            _body, mesh=mesh,
            in_specs=(PartitionSpec("core"),) * len(in_names),
            out_specs=(PartitionSpec("core"),) * len(out_names),
            check_rep=False))
        self._put = lambda a: jax.device_put(a, self.sharding)

    def put(self, arr):
        return self._put(arr)

    def __call__(self, gin):
        outs = self.jitted(*[gin[n] for n in self.in_names])
        return {n: np.asarray(o) for n, o in zip(self.out_names, outs)}


# --------------------------------------------------------------------------
# kernel entry
# --------------------------------------------------------------------------

def _graph_key(src, dst):
    s = np.ascontiguousarray(src)
    d = np.ascontiguousarray(dst)
    return (s.shape[0], zlib.adler32(s.tobytes()), zlib.adler32(d.tobytes()))


def _kernel_device(feature, src, dst, W_, b):
    key = _graph_key(src, dst)
    if _STATE.get("graph_key") != key:
        g, meta = _prep_graph(src, dst)
        _STATE["runner"] = _Runner(_build_nc(meta))
        runner = _STATE["runner"]
        _STATE["graph"] = {k: runner.put(v) for k, v in g.items()}
        _STATE["graph_key"] = key
        _STATE["reps1"] = runner.put(
            np.tile(np.array([[1]], np.uint32), (N_CORES, 1)))
    runner = _STATE["runner"]

    featT = np.zeros((N_CORES, 128, NPAD), np.float16)
    f16 = np.asarray(feature, np.float16).reshape(N_CORES, NS, D)
    featT[:, :, :NS] = f16.transpose(0, 2, 1)
    wt = np.ascontiguousarray(np.asarray(W_).T, dtype=np.float16)
    gin = {
        **_STATE["graph"],
        "featT": featT.reshape(N_CORES * 128, NPAD),
        "wt": np.ascontiguousarray(np.tile(wt, (N_CORES, 1))),
        "bias": np.ascontiguousarray(
            np.tile(np.asarray(b, np.float16)[None, :], (N_CORES, 1))),
        "reps": _STATE["reps1"],
    }
    out = runner(gin)["out"]
    _STATE["last_gin"] = {k: (v if k in ("gidx", "slots", "cnts", "reps")
                              else runner.put(v)) for k, v in gin.items()}
    return np.ascontiguousarray(out.reshape(N_NODES, D))


def _kernel_numpy(feature, src, dst, W_, b):
    """Host fallback (correct for any shapes)."""
    feature = np.asarray(feature, dtype=np.float32)
    agg = np.zeros_like(feature)
    np.add.at(agg, np.asarray(dst).astype(np.int64),
              feature[np.asarray(src).astype(np.int64)])
    return agg @ np.asarray(W_, np.float32).T + np.asarray(b, np.float32)


def kernel(feature, src, dst, W, b):
    feature = np.asarray(feature)
    if (feature.shape != (N_NODES, D) or np.asarray(W).shape != (D, D)):
        return _kernel_numpy(feature, src, dst, W, b)
    try:
        return _kernel_device(feature, src, dst, W, b)
    except Exception:
        import traceback
        traceback.print_exc()
        return _kernel_numpy(feature, src, dst, W, b)
